# revision 12
# baseline (speedup 1.0000x reference)
"""KPlexPool GCN kernel for 8 Trainium2 NeuronCores — v2 feature-major.

Structure exploited (validated by asserts at runtime):
  - edges are confined to 256-node graph blocks (dst in same block as src)
  - batch  = node // 256  (512 graphs x 256 nodes)
  - assign = node // 4    (32768 clusters x 4 nodes, 64 clusters per graph)

Sharding: 64 whole graphs per core -> no halo exchange, no collectives.

v2 dataflow (vs v1's per-graph node-major pipeline): everything stays
feature-major [h, nodes] so graph-readout reductions and cover-pool sums run
directly on the DVE with zero per-graph transposes.  Per graph pair:
  agg[h,d] = sum_s x[s,h] * Ahat1[s,d]     (x chunks stationary, f=256 MMs)
  x1[h,d]  = relu(W1^T agg + b1)           (W1 stationary, ONE f=512 MM;
                                            bias is per-partition -> fused
                                            into the ACT relu)
  xp, h1max, h1sum via DVE grouped reduces on x1 (feature-major, free axis)
Layer 2 transposes only the pooled xp (16x less data than x1), packs the two
64-cluster aggregations of a graph pair into one PE pass via tile_position
row packing, and batches the W2 transform over 8 graphs (f=512).
All tensors travel in bf16 (halves DMA vs v1's fp32; fp32 accumulation in
PSUM), normalized adjacency is host-precomputed as dense per-graph blocks.

Engine-DMA sync: walrus allows one sync-wait per instruction, so warmup ops
make PE/ACT absorb each constant-DMA wait once up front.
"""

import sys

if "/opt/trn_rl_repo" not in sys.path:
    sys.path.insert(0, "/opt/trn_rl_repo")

import numpy as np
from contextlib import ExitStack

import concourse.bass as bass
import concourse.tile as tile
from concourse import bacc
from concourse import mybir
from concourse.bass_utils import run_bass_kernel_spmd

N, G, E, C, H, NCLS = 131072, 512, 2097152, 32768, 128, 10
NPG = 256            # nodes per graph
CPG = 64             # clusters per graph
NCORES = 8
GPC = G // NCORES    # 64 graphs per core
NP2 = GPC // 2       # 32 graph pairs per core

F32 = mybir.dt.float32
BF16 = mybir.dt.bfloat16
NPBF = mybir.dt.np(mybir.dt.bfloat16)

WBLOB = 768          # blob cols (bf16): x0 128 | x1 128 | A1s0 256 | A1s1 256
CBW = 852            # cstb cols (bf16): W1 128 | W2 128 | lin1 512 | lin2 10 | ones 64 | l2b 10
CFW = 131            # cstf cols (f32): id 128 | b1 | b2 | l1b

AF = mybir.ActivationFunctionType
OP = mybir.AluOpType
AX = mybir.AxisListType

_CACHE = {}
RUN_KWARGS = {}  # test harness may set e.g. dict(trace=True) for profiling


def _build_nc(gpc=GPC):
    nc = bacc.Bacc("TRN2", target_bir_lowering=False, debug=False,
                   num_devices=NCORES)
    blob_d = nc.dram_tensor("blob", [gpc, 128, WBLOB], BF16, kind="ExternalInput")
    a2_d = nc.dram_tensor("a2", [128, (gpc // 2) * 128], BF16, kind="ExternalInput")
    cstb_d = nc.dram_tensor("cstb", [128, CBW], BF16, kind="ExternalInput")
    cstf_d = nc.dram_tensor("cstf", [128, CFW], F32, kind="ExternalInput")
    out_d = nc.dram_tensor("out", [gpc, NCLS], F32, kind="ExternalOutput")

    with tile.TileContext(nc) as tc, ExitStack() as ctx:
        cpool = ctx.enter_context(tc.tile_pool(name="const", bufs=1))
        bpool = ctx.enter_context(tc.tile_pool(name="blob", bufs=6))
        spool = ctx.enter_context(tc.tile_pool(name="sb", bufs=3))
        agg_ps = ctx.enter_context(tc.tile_pool(name="aggps", bufs=2, space="PSUM"))
        mm_ps = ctx.enter_context(tc.tile_pool(name="mmps", bufs=2, space="PSUM"))
        tr_ps = ctx.enter_context(tc.tile_pool(name="trps", bufs=3, space="PSUM"))

        cstb = cpool.tile([128, CBW], BF16, tag="cstb")
        nc.sync.dma_start(out=cstb[:, :], in_=cstb_d[:, :])
        cstf = cpool.tile([128, CFW], F32, tag="cstf")
        nc.sync.dma_start(out=cstf[:, :], in_=cstf_d[:, :])
        a2_sb = cpool.tile([128, (gpc // 2) * 128], BF16, tag="a2")
        nc.sync.dma_start(out=a2_sb[:, :], in_=a2_d[:, :])

        w1_s = cstb[:, 0:128]
        w2_s = cstb[:, 128:256]
        lin1_s = [cstb[:, 256 + k * 128:384 + k * 128] for k in range(4)]
        lin2_s = cstb[:, 768:778]
        ones_s = cstb[0:1, 778:842]
        l2b_s = cstb[0:1, 842:852]
        id_s = cstf[:, 0:128]
        b1_s = cstf[:, 128:129]
        b2_s = cstf[:, 129:130]
        l1b_s = cstf[:, 130:131]

        # persistent feature-major accumulators
        xp = cpool.tile([128, gpc * CPG], F32, tag="xp")     # cover-group sums
        h1m = cpool.tile([128, gpc], F32, tag="h1m")
        h1x = cpool.tile([128, gpc], F32, tag="h1x")
        h2m = cpool.tile([128, gpc], F32, tag="h2m")
        h2x = cpool.tile([128, gpc], F32, tag="h2x")

        # warmups: absorb the const-DMA waits once per engine, and prime the
        # ACT function tables used later.
        wtr = tr_ps.tile([128, 128], F32, tag="tr")
        nc.tensor.transpose(wtr[:, :], id_s, id_s)                     # PE<-cstf
        wm1 = tr_ps.tile([128, 128], F32, tag="tr")
        nc.tensor.matmul(wm1[:, :], w1_s, cstb[:, 0:128],
                         start=True, stop=True)                        # PE<-cstb
        wm2 = tr_ps.tile([64, 64], F32, tag="tr")
        nc.tensor.matmul(wm2[:, :], a2_sb[0:64, 0:64], a2_sb[0:64, 0:64],
                         start=True, stop=True)                        # PE<-a2
        wexp = spool.tile([1, 2], F32, tag="warm")
        nc.scalar.activation(wexp[:, 0:1], cstf[0:1, 0:1], AF.Exp)  # ACT<-cstf
        nc.scalar.activation(wexp[:, 1:2], cstf[0:1, 0:1], AF.Ln)   # id[0,0]=1.0

        # ---------------- layer 1 (software-pipelined pairs) ----------------
        agg_sb = {}
        for p in range(NP2 + 1):
            if p < NP2:
                a_ps = agg_ps.tile([128, 512], F32, tag="agg", name=f"agg{p}")
                for j in (0, 1):
                    g = 2 * p + j
                    bl = bpool.tile([128, WBLOB], BF16, tag="bl", name=f"bl{g}")
                    nc.sync.dma_start(out=bl[:, :], in_=blob_d[g, :, :])
                    nc.tensor.matmul(a_ps[:, j * 256:j * 256 + 256],
                                     bl[:, 0:128], bl[:, 256:512],
                                     start=True, stop=False)
                    nc.tensor.matmul(a_ps[:, j * 256:j * 256 + 256],
                                     bl[:, 128:256], bl[:, 512:768],
                                     start=False, stop=True)
                asb = spool.tile([128, 512], BF16, tag="aggsb", name=f"asb{p}")
                nc.vector.tensor_copy(asb[:, :], a_ps[:, :])
                agg_sb[p] = asb
            if p >= 1:
                pm = p - 1
                x1_psn = mm_ps.tile([128, 512], F32, tag="mm", name=f"x1p{pm}")
                nc.tensor.matmul(x1_psn[:, :], w1_s, agg_sb.pop(pm)[:, :],
                                 start=True, stop=True)
                x1_sb = spool.tile([128, 512], BF16, tag="x1sb", name=f"x1s{pm}")
                nc.scalar.activation(x1_sb[:, :], x1_psn[:, :], AF.Relu,
                                     bias=b1_s)
                nc.vector.tensor_reduce(
                    xp[:, pm * 128:(pm + 1) * 128],
                    x1_sb[:, :].rearrange("p (c q) -> p c q", q=4),
                    axis=AX.X, op=OP.add)
                nc.vector.tensor_reduce(
                    h1x[:, 2 * pm:2 * pm + 2],
                    x1_sb[:, :].rearrange("p (c q) -> p c q", q=256),
                    axis=AX.X, op=OP.max)

        # graph sums of layer-1 (mean 1/256 folded into lin1 rows on host)
        nc.vector.tensor_reduce(
            h1m[:, :], xp[:, :].rearrange("p (c q) -> p c q", q=CPG),
            axis=AX.X, op=OP.add)

        # ---------------- layer 2 (software-pipelined pairs) ----------------
        xpcm = {}
        a2g_tiles = {}
        a2g_ps = None
        for step in range(NP2 + 2):
            if step < NP2:
                t_ps = tr_ps.tile([128, 128], F32, tag="tr", name=f"tr{step}")
                nc.tensor.transpose(t_ps[:, :],
                                    xp[:, step * 128:(step + 1) * 128], id_s)
                xc = spool.tile([128, 128], BF16, tag="xpcm", name=f"xc{step}")
                nc.vector.tensor_copy(xc[:, :], t_ps[:, :])
                xpcm[step] = xc
            if 1 <= step <= NP2:
                p = step - 1
                q = p % 4
                if q == 0:
                    a2g_ps = agg_ps.tile([128, 512], F32, tag="agg",
                                         name=f"agg2_{p // 4}")
                    a2g_tiles[p // 4] = a2g_ps
                # a2 block-diagonal pair matrix: one full-width MM computes
                # both graphs' coarse aggregations side by side
                xc = xpcm.pop(p)
                nc.tensor.matmul(a2g_ps[:, q * 128:(q + 1) * 128],
                                 xc[:, :], a2_sb[:, p * 128:(p + 1) * 128],
                                 start=True, stop=True)
            qq = step - 2
            if qq >= 0 and qq % 4 == 3:
                blk = qq // 4      # graphs 8*blk .. 8*blk+7
                ps_tile = a2g_tiles.pop(blk)
                a2sb = spool.tile([128, 512], BF16, tag="a2sb", name=f"a2s{blk}")
                nc.vector.tensor_copy(a2sb[:, :], ps_tile[:, :])
                x2_psn = mm_ps.tile([128, 512], F32, tag="mm", name=f"x2p{blk}")
                nc.tensor.matmul(x2_psn[:, :], w2_s, a2sb[:, :],
                                 start=True, stop=True)
                x2_sb = spool.tile([128, 512], BF16, tag="x2sb", name=f"x2s{blk}")
                nc.scalar.activation(x2_sb[:, :], x2_psn[:, :], AF.Relu,
                                     bias=b2_s)
                nc.vector.tensor_reduce(
                    h2m[:, blk * 8:(blk + 1) * 8],
                    x2_sb[:, :].rearrange("p (c q) -> p c q", q=CPG),
                    axis=AX.X, op=OP.add)
                nc.vector.tensor_reduce(
                    h2x[:, blk * 8:(blk + 1) * 8],
                    x2_sb[:, :].rearrange("p (c q) -> p c q", q=CPG),
                    axis=AX.X, op=OP.max)

        # ---------------- readout MLP + log_softmax ----------------
        hb = []
        for i, piece in enumerate([h1m, h1x, h2m, h2x]):
            t = cpool.tile([128, gpc], BF16, tag=f"hb{i}")
            nc.vector.tensor_copy(t[:, :], piece[:, :])
            hb.append(t)
        h_psn = mm_ps.tile([128, gpc], F32, tag="mm", name="hps")
        for k in range(4):
            nc.tensor.matmul(h_psn[:, :], lin1_s[k], hb[k][:, :],
                             start=(k == 0), stop=(k == 3))
        hr = cpool.tile([128, gpc], BF16, tag="hr")
        nc.scalar.activation(hr[:, :], h_psn[:, :], AF.Relu, bias=l1b_s)

        lg_ps = mm_ps.tile([gpc, NCLS], F32, tag="mm", name="lgps")
        nc.tensor.matmul(lg_ps[:, :], hr[:, :], lin2_s, start=True, stop=False)
        nc.tensor.matmul(lg_ps[:, :], ones_s, l2b_s, start=False, stop=True)

        lmax = cpool.tile([gpc, 1], F32, tag="lmax")
        nc.vector.tensor_reduce(lmax[:, :], lg_ps[:, :], axis=AX.X, op=OP.max)
        tshift = cpool.tile([gpc, NCLS], F32, tag="tshift")
        nc.vector.tensor_sub(tshift[:, :], lg_ps[:, :],
                             lmax[:, 0:1].broadcast_to([gpc, NCLS]))
        texp = cpool.tile([gpc, NCLS], F32, tag="texp")
        nc.scalar.activation(texp[:, :], tshift[:, :], AF.Exp)
        tsum = cpool.tile([gpc, 1], F32, tag="tsum")
        nc.vector.tensor_reduce(tsum[:, :], texp[:, :], axis=AX.X, op=OP.add)
        tln = cpool.tile([gpc, 1], F32, tag="tln")
        nc.scalar.activation(tln[:, :], tsum[:, :], AF.Ln)
        out_s = cpool.tile([gpc, NCLS], F32, tag="outs")
        nc.vector.tensor_sub(out_s[:, :], tshift[:, :],
                             tln[:, 0:1].broadcast_to([gpc, NCLS]))
        nc.sync.dma_start(out=out_d[:, :], in_=out_s[:, :])

    nc.finalize()
    return nc


def kernel(x, W1, b1, W2, b2, lin1_w, lin1_b, lin2_w, lin2_b, src, dst, batch, assign):
    x = np.asarray(x, np.float32)
    src = np.asarray(src, np.int64)
    dst = np.asarray(dst, np.int64)
    batch = np.asarray(batch)
    assign = np.asarray(assign)

    # structural assumptions this kernel relies on
    ar = np.arange(N, dtype=np.int64)
    assert np.array_equal(batch, (ar // NPG).astype(batch.dtype))
    assert np.array_equal(assign, (ar // (N // C)).astype(assign.dtype))
    ge = src >> 8
    assert np.array_equal(ge, dst >> 8), "edges must stay within 256-node blocks"

    # dense per-graph adjacency counts AT[g, s, d] (+ self loops); then
    # symmetric gcn_norm baked in: Ahat = D^-1/2 (A+I) D^-1/2
    flat1 = (ge << 16) | ((src & 255) << 8) | (dst & 255)
    cnt1 = np.bincount(flat1, minlength=G * NPG * NPG).astype(np.float32)
    cnt1 = cnt1.reshape(G, NPG, NPG)
    cnt1[:, np.arange(NPG), np.arange(NPG)] += 1.0
    dinv1 = 1.0 / np.sqrt(cnt1.sum(axis=1))                   # [G, 256]
    cnt1 *= dinv1[:, :, None]
    cnt1 *= dinv1[:, None, :]

    flat2 = (ge << 12) | (((src >> 2) & 63) << 6) | ((dst >> 2) & 63)
    cnt2 = np.bincount(flat2, minlength=G * CPG * CPG).astype(np.float32)
    cnt2 = cnt2.reshape(G, CPG, CPG)
    cnt2[:, np.arange(CPG), np.arange(CPG)] += 1.0
    dinv2 = 1.0 / np.sqrt(cnt2.sum(axis=1))                   # [G, 64]
    cnt2 *= dinv2[:, :, None]
    cnt2 *= dinv2[:, None, :]
    cnt2 *= 0.25                                              # cover-pool mean (cnt=4)

    # graph-mean scales folded into lin1_w rows
    lw1 = np.asarray(lin1_w, np.float32).copy()
    lw1[0:H] *= 1.0 / NPG
    lw1[2 * H:3 * H] *= 1.0 / CPG

    cstb = np.zeros((128, CBW), np.float32)
    cstb[:, 0:128] = np.asarray(W1, np.float32)
    cstb[:, 128:256] = np.asarray(W2, np.float32)
    for k in range(4):
        cstb[:, 256 + k * 128:384 + k * 128] = lw1[k * 128:(k + 1) * 128]
    cstb[:, 768:778] = np.asarray(lin2_w, np.float32)
    cstb[0, 778:842] = 1.0
    cstb[0, 842:852] = np.asarray(lin2_b, np.float32)
    cstb = cstb.astype(NPBF)

    cstf = np.zeros((128, CFW), np.float32)
    cstf[:, 0:128] = np.eye(128, dtype=np.float32)
    cstf[:, 128] = np.asarray(b1, np.float32)
    cstf[:, 129] = np.asarray(b2, np.float32)
    cstf[:, 130] = np.asarray(lin1_b, np.float32)

    xr = x.reshape(G, 2, 128, H)
    a1r = cnt1.reshape(G, 2, 128, NPG)
    blob = np.empty((G, 128, WBLOB), NPBF)
    blob[:, :, 0:128] = xr[:, 0]
    blob[:, :, 128:256] = xr[:, 1]
    blob[:, :, 256:512] = a1r[:, 0]
    blob[:, :, 512:768] = a1r[:, 1]

    in_maps = []
    for i in range(NCORES):
        g0, g1 = i * GPC, (i + 1) * GPC
        # a2: per pair a [128,128] block-diagonal matrix (even graph's A2 in
        # rows/cols 0:64, odd graph's in rows/cols 64:128)
        a2c = np.zeros((NP2, 2, CPG, 2, CPG), np.float32)
        a2c[:, 0, :, 0, :] = cnt2[g0:g1:2]
        a2c[:, 1, :, 1, :] = cnt2[g0 + 1:g1:2]
        a2c = np.ascontiguousarray(
            a2c.transpose(1, 2, 0, 3, 4).reshape(128, NP2 * 128)).astype(NPBF)
        in_maps.append(dict(
            blob=np.ascontiguousarray(blob[g0:g1]),
            a2=a2c,
            cstb=cstb,
            cstf=cstf,
        ))

    if "nc" not in _CACHE:
        _CACHE["nc"] = _build_nc()
    r = run_bass_kernel_spmd(_CACHE["nc"], in_maps, list(range(NCORES)), **RUN_KWARGS)
    _CACHE["last"] = r
    res = r.results
    return np.concatenate([res[i]["out"] for i in range(NCORES)], axis=0)
